# revision 6
# baseline (speedup 1.0000x reference)
"""CGMM layer-0 forward on 8 Trainium2 NeuronCores — v3.

Math: lik_graph[s, :] = sum_m count[s, m] * L[m, :] with L a (32, 16)
table computed from (B, Pi); count = per-graph label histogram.

v3 design (vs the radix-packed baseline):
  - Labels x = 11*t + m ship as XL = m and section indicators
    XSec[slot, col, j] = [t == j]; no radix scales.
  - H planes: 11 tensor_scalar is_equal ops (DVE 4x / Pool), plane-major
    layout [slot, m, col] so writes are stride-1.
  - Counts via ONE tiny PE matmul per column: lhsT = H[:, :, w]
    ([128, 11], strided free), rhs = XSec[:, w, :] ([128, 3]) ->
    psum [11, 3] = counts split by section.  No XS multiply, no decode.
    Mirror columns psum-accumulate onto their parent's block; the host
    puts the 16 largest graphs at cols 480..495 (bank 3, wave 2).
  - Two column waves so count-mms pipeline under later planes.
  - L table: Z/ZP contractions per g; ZP in 3-term form (ebp =
    expB*(B+Pi), w2' = wb*(-ln sumB)) so no eb/lnw hop; ZP/Z via the
    divide ALU; Ltn expanded to the block-diagonal [128, 3, 64] final
    lhsT by EE selection matmuls (blocks at partitions 32u).
  - Final: 3 accumulating matmuls out[64, 160] += LtnW[:,t,:]^T @
    CNTS[:,:,t]; one psum->SBUF copy; one small output DMA.
Column map: column w holds host-assigned graph (16 largest at 480..495);
output row p = 16*u + g for column w = 160*u + c.
"""

import math

import numpy as np

N_NODES = 500_000
N_GRAPHS = 5_000
C = 16
M = 32
G = 16
N_CORES = 8
GPC = N_GRAPHS // N_CORES   # 625 graphs per core
R = 11                      # plane alphabet (m in [0, 11))
NSEC = 3                    # digit sections (t in [0, 3))
NBG = 160                   # columns per psum bank
WREAL = 4 * NBG             # 640 matmulled columns (625 graphs + 15 pad)
OVW = 16                    # mirror columns for graphs > 127 nodes
W = WREAL + OVW             # 656 total columns
PARENT0 = 480               # parent cols for mirrors: 480..495 (bank 3)
TCAP = 128
TUSE = 127
PAD_LABEL = 64.0

WSPLIT = 320                # wave boundary (banks 0,1 | banks 2,3 + mirrors)
DVE_PLANES = 6              # planes 0..5 on DVE, 6..10 on Pool


def _build_nc():
    import concourse.bass as bass
    import concourse.bacc as bacc
    import concourse.tile as tile
    import concourse.mybir as mybir

    fp32 = mybir.dt.float32
    bf16 = mybir.dt.bfloat16
    Alu = mybir.AluOpType
    Act = mybir.ActivationFunctionType

    nc = bacc.Bacc("TRN2", target_bir_lowering=False, debug=False)

    from concourse.hw_specs import get_activation_tables
    tabs = get_activation_tables(nc.m.arch)
    need = {Act.Exp, Act.Ln, Act.Copy}
    act_set_id = next(i for i, v in enumerate(tabs.values()) if need <= v)

    xl_d = nc.dram_tensor("xl", [TCAP, W], bf16, kind="ExternalInput").ap()
    xsa_d = nc.dram_tensor("xsa", [TCAP, WSPLIT * NSEC], bf16,
                           kind="ExternalInput").ap()
    xsb_d = nc.dram_tensor("xsb", [TCAP, (W - WSPLIT) * NSEC], bf16,
                           kind="ExternalInput").ap()
    par_d = nc.dram_tensor("par", [C, G * M + G], fp32, kind="ExternalInput").ap()
    # rows 0..15 cols 0..511: bp = (B+Pi) g-major; cols 512..: EE [32, 12*128]
    bpee_d = nc.dram_tensor("bpee", [M, G * M + 12 * TCAP], bf16,
                            kind="ExternalInput").ap()
    out_d = nc.dram_tensor("out", [4 * G, 192], fp32, kind="ExternalOutput").ap()
    idx_d = nc.dram_tensor("idx", [128, 4], mybir.dt.int16,
                           kind="ExternalInput").ap()

    with tile.TileContext(nc) as tc:
        with (
            tc.tile_pool(name="main", bufs=1) as main,
            tc.tile_pool(name="psA", bufs=1, space="PSUM") as psA,
            tc.tile_pool(name="psB", bufs=1, space="PSUM") as psB,
            tc.tile_pool(name="psC", bufs=1, space="PSUM") as psC,
            tc.tile_pool(name="psD", bufs=1, space="PSUM") as psD,
            tc.tile_pool(name="psE", bufs=1, space="PSUM") as psE,
            tc.tile_pool(name="psO", bufs=1, space="PSUM") as psO,
        ):
            # ---- input DMAs ----
            XL = main.tile([TCAP, W], bf16)
            nc.sync.dma_start(out=XL, in_=xl_d)              # SP#1
            Par = main.tile([C, G * M + G], fp32)
            nc.gpsimd.dma_start(out=Par, in_=par_d)          # Pool#1
            XSa = main.tile([TCAP, WSPLIT, NSEC], bf16)
            nc.sync.dma_start(                               # SP#2
                out=XSa, in_=xsa_d.rearrange("p (w n) -> p w n", n=NSEC))
            BPEE = main.tile([M, G * M + 12 * TCAP], bf16)
            nc.sync.dma_start(out=BPEE, in_=bpee_d)          # SP#3
            XSb = main.tile([TCAP, W - WSPLIT, NSEC], bf16)
            nc.sync.dma_start(                               # SP#4
                out=XSb, in_=xsb_d.rearrange("p (w n) -> p w n", n=NSEC))
            IDX = main.tile([128, 4], mybir.dt.int16)
            nc.sync.dma_start(out=IDX, in_=idx_d)            # SP#5 (tiny)
            bp = BPEE[0:C, 0 : G * M]
            EE = BPEE[:, G * M :].rearrange("p (j x) -> p j x", x=TCAP)

            Bt = Par[:, 0 : G * M]
            Pit = Par[:, G * M : G * M + G]

            onesc = main.tile([C, 1], fp32)
            nc.gpsimd.memset(onesc, 1.0)
            onesm1 = main.tile([1, M], fp32)
            nc.gpsimd.memset(onesm1, 1.0)
            CNTS = main.tile([TCAP, NBG, NSEC], bf16)
            nc.gpsimd.memset(CNTS, 0.0)
            OS = main.tile([TCAP, 192], fp32)
            nc.gpsimd.memset(OS, 0.0)
            OSZ = main.tile([4 * G, 192], fp32)
            nc.gpsimd.memset(OSZ, 0.0)

            # ---- Act: one explicit table load covering Exp+Ln+Copy ----
            ld = mybir.InstLoadActFuncSet(
                name=nc.get_next_instruction_name(), ins=[], outs=[],
                act_func_set_id=act_set_id)
            ld.engine = mybir.EngineType.Activation
            nc.scalar.add_instruction(ld)

            expPi = main.tile([C, G], fp32)
            nc.scalar.activation(expPi, Pit, Act.Exp)
            expB = main.tile([C, G * M], bf16)
            nc.scalar.activation(expB, Bt, Act.Exp)

            ps_small = psE.tile([M, 4, G], fp32)
            spi = ps_small[0:1, 0, :]
            nc.tensor.matmul(spi, onesc[:, 0:1], expPi, start=True, stop=True)
            lnspi = main.tile([1, G], fp32)
            with tc.high_priority():
                nc.scalar.activation(lnspi, spi, Act.Ln)

            # ---- planes (plane-major H) + per-column count matmuls ----
            H = main.tile([TCAP, R, W], bf16)
            CNT01 = psA.tile([43, NBG, NSEC], fp32)
            CNT23 = psB.tile([43, NBG, NSEC], fp32)
            # zero rows 11..31 (copy reads them); runs pre-XL on idle DVE
            nc.vector.memset(CNT01, 0.0)
            nc.vector.memset(CNT23, 0.0)

            def cnt_out(u, c):
                tile_ = CNT01 if u < 2 else CNT23
                off = 32 * (u % 2)
                return tile_[off : off + R, c, :]

            def planes(lo, hi, dve_n):
                for m in range(R):
                    eng = nc.vector if m < dve_n else nc.gpsimd
                    eng.tensor_scalar(
                        out=H[:, m, lo:hi], in0=XL[:, lo:hi],
                        scalar1=float(m), scalar2=0.0,
                        op0=Alu.is_equal, op1=Alu.add,
                    )

            def xsec(w):
                if w < WSPLIT:
                    return XSa[:, w, :]
                return XSb[:, w - WSPLIT, :]

            def count_mms(lo, hi):
                for w in range(lo, hi):
                    u, c = w // NBG, w % NBG
                    is_parent = PARENT0 <= w < PARENT0 + OVW
                    nc.tensor.matmul(cnt_out(u, c), H[:, :, w], xsec(w),
                                     start=True, stop=not is_parent)
                    if is_parent:
                        # mirror column accumulates onto the parent block;
                        # keep the pair consecutive (one pending psum group)
                        mw = WREAL + (w - PARENT0)
                        nc.tensor.matmul(cnt_out(u, c), H[:, :, mw],
                                         xsec(mw), start=False, stop=True)

            # wave 1: banks 0, 1
            planes(0, WSPLIT, 7)
            count_mms(0, WSPLIT)

            # ---- L-chain smalls on DVE (high priority vs planes) ----
            sumB = main.tile([C, G], bf16)
            rsumB = main.tile([C, G], fp32)
            wb = main.tile([C, G], bf16)
            ebp = main.tile([C, G * M], bf16)
            with tc.high_priority():
                with nc.allow_low_precision(reason="bf16 sumB feeds a ratio"):
                    nc.vector.tensor_reduce(
                        sumB, expB.rearrange("c (g m) -> c g m", m=M),
                        mybir.AxisListType.X, Alu.add)
                nc.vector.reciprocal(rsumB, sumB)
                with nc.allow_low_precision(reason="bf16 posterior weights"):
                    nc.vector.tensor_tensor(out=wb, in0=expPi, in1=rsumB,
                                            op=Alu.mult)
                with nc.allow_low_precision(reason="bf16 table build"):
                    nc.vector.tensor_tensor(out=ebp, in0=expB, in1=bp,
                                            op=Alu.mult)

            # Z matmuls
            z32 = ps_small[:, 1, :]
            zp32 = ps_small[:, 2, :]
            lnspiM = ps_small[:, 3, :]
            for g in range(G):
                bg = expB[:, g * M : (g + 1) * M]
                nc.tensor.matmul(z32[:, g : g + 1], bg, wb[:, g : g + 1],
                                 start=True, stop=True)
            # 1/Z early (reciprocal may read psum; result SBUF, off-critical)
            rz = main.tile([M, G], fp32)
            with tc.high_priority():
                nc.vector.reciprocal(rz, z32)

            # wave 2: banks 2, 3 (incl. mirror parents at 480..495)
            planes(WSPLIT, W, 5)

            # Act: single Ln-table swap, then the two tiny lns
            lnsumB = main.tile([C, G], fp32)
            with tc.high_priority():
                nc.scalar.activation(lnsumB, sumB, Act.Ln)

            w2n = main.tile([C, G], bf16)
            with tc.high_priority():
                with nc.allow_low_precision(reason="bf16 posterior weights"):
                    nc.vector.scalar_tensor_tensor(
                        out=w2n, in0=lnsumB, scalar=-1.0, in1=wb,
                        op0=Alu.mult, op1=Alu.mult)
            for g in range(G):
                bg = expB[:, g * M : (g + 1) * M]
                nc.tensor.matmul(zp32[:, g : g + 1],
                                 ebp[:, g * M : (g + 1) * M],
                                 wb[:, g : g + 1], start=True, stop=False)
                nc.tensor.matmul(zp32[:, g : g + 1], bg, w2n[:, g : g + 1],
                                 start=False, stop=True)
            nc.tensor.matmul(lnspiM, onesm1, lnspi, start=True, stop=True)

            count_mms(WSPLIT, WREAL)

            # ---- L tail: Lq = ZP/Z; Ltn = lnspiM - Lq; EE expansion ----
            Lq = main.tile([M, G], fp32)
            with tc.high_priority():
                nc.vector.tensor_tensor(out=Lq, in0=zp32, in1=rz, op=Alu.mult)
            Ltn = main.tile([M, G], bf16)
            with tc.high_priority():
                with nc.allow_low_precision(reason="bf16 L table"):
                    nc.vector.tensor_tensor(out=Ltn, in0=lnspiM, in1=Lq,
                                            op=Alu.subtract)
            LtnWP = psO.tile([TCAP, NSEC, 4, G], fp32)
            for t in range(NSEC):
                for u in range(4):
                    nc.tensor.matmul(LtnWP[:, t, u, :], EE[:, 4 * t + u, :],
                                     Ltn, start=True, stop=True)
            LtnW = main.tile([TCAP, NSEC, 4 * G], bf16)
            with tc.high_priority():
                with nc.allow_low_precision(reason="bf16 L table"):
                    nc.vector.tensor_scalar(
                        out=LtnW, in0=LtnWP.rearrange("p t u g -> p t (u g)"),
                        scalar1=0.0, scalar2=0.0, op0=Alu.add, op1=Alu.add)

            # ---- CNTS copies: two [43, .] copies (banks live at 32u) ----
            with nc.allow_low_precision(reason="counts<128 exact"):
                nc.scalar.copy(CNTS[0:43, :, :], CNT01)
            with nc.allow_low_precision(reason="counts<128 exact"):
                nc.scalar.copy(CNTS[64:107, :, :], CNT23)

            # ---- final contraction + output ----
            OUT = psE.tile([4 * G, NBG], fp32)
            for t in range(NSEC):
                nc.tensor.matmul(OUT, LtnW[:, t, :], CNTS[:, :, t],
                                 start=(t == 0), stop=(t == NSEC - 1))
            with tc.high_priority():
                nc.vector.tensor_scalar(out=OS[0 : 4 * G, 0 : NBG // 2],
                                        in0=OUT[:, 0 : NBG // 2],
                                        scalar1=0.0, scalar2=0.0,
                                        op0=Alu.add, op1=Alu.add)
            with tc.high_priority():
                nc.scalar.copy(OS[0 : 4 * G, NBG // 2 : NBG],
                               OUT[:, NBG // 2 :])
            # zero the DRAM output on the SAME SWDGE ring as the scatter:
            # ring entries fire in order, so the zero always lands first
            # (an HWDGE zero raced the triggered scatter on real HW)
            nc.gpsimd.dma_start(out=out_d, in_=OSZ)
            dma_sem = nc.alloc_semaphore("out_scatter_dma")
            nc.gpsimd.dma_scatter_add(
                out_ap=out_d,
                in_ap=OS.rearrange("p (a b) -> p a b", a=1),
                idxs_ap=IDX, num_idxs=4 * G, num_idxs_reg=4 * G,
                elem_size=192, elem_step=192, prepare_only=True,
                sem=dma_sem)
            nc.gpsimd.trigger_dma(count=None)

    nc.compile()
    return nc


def _host_pack(x, batch):
    """Pack per-core node labels into the v3 layout.

    Returns (XLs, XSecs, orders, cmaps): XL [TCAP, W] bf16 label planes,
    XSec [TCAP, W*NSEC] bf16 section indicators, plus per-core sort order
    and column assignment (col of sorted-graph i)."""
    import ml_dtypes

    sizes = np.bincount(batch, minlength=N_GRAPHS)
    T = int(sizes.max())
    assert T - TUSE <= TUSE, "graph overflow exceeds one mirror column"
    Tp = max(T, TUSE)
    xv = x.astype(np.int64)
    m_all = (xv % R).astype(np.float32)
    t_all = (xv // R).astype(np.int64)

    # padded per-graph grids
    xp = np.full((N_GRAPHS, Tp), PAD_LABEL, dtype=np.float32)
    tp = np.full((N_GRAPHS, Tp), -1, dtype=np.int64)
    mask = np.arange(Tp)[None, :] < sizes[:, None]
    xp[mask] = m_all          # batch sorted -> row-major True order matches x
    tp[mask] = t_all

    # column map: sorted-graph i<16 -> PARENT0+i; rest fill other cols
    rest_cols = np.concatenate([np.arange(0, PARENT0),
                                np.arange(PARENT0 + OVW, WREAL)])
    cmap = np.empty(GPC, dtype=np.int64)
    cmap[:OVW] = PARENT0 + np.arange(OVW)
    cmap[OVW:] = rest_cols[: GPC - OVW]

    XLs, XSecs, orders = [], [], []
    for i in range(N_CORES):
        s = sizes[i * GPC : (i + 1) * GPC]
        order = np.argsort(-s, kind="stable")
        orders.append(order)
        xs_ = xp[i * GPC : (i + 1) * GPC][order]   # [GPC, Tp] size-desc
        ts_ = tp[i * GPC : (i + 1) * GPC][order]
        n_ov = int((s > TUSE).sum())
        assert n_ov <= OVW, f"core {i}: {n_ov} oversized graphs > {OVW}"

        XL = np.full((TCAP, W), PAD_LABEL, dtype=np.float32)
        SEC = np.zeros((TCAP, W, NSEC), dtype=np.float32)
        XL[:TUSE, cmap] = xs_[:, :TUSE].T
        tm = ts_[:, :TUSE].T                        # [TUSE, GPC]
        for t in range(NSEC):
            SEC[:TUSE, cmap, t] = (tm == t)
        if n_ov and T > TUSE:
            ovt = T - TUSE
            XL[:ovt, WREAL : WREAL + n_ov] = xs_[:n_ov, TUSE:T].T
            to = ts_[:n_ov, TUSE:T].T
            for t in range(NSEC):
                SEC[:ovt, WREAL : WREAL + n_ov, t] = (to == t)
        XLs.append(np.ascontiguousarray(XL.astype(ml_dtypes.bfloat16)))
        XSecs.append(np.ascontiguousarray(
            SEC.reshape(TCAP, W * NSEC).astype(ml_dtypes.bfloat16)))
    return XLs, XSecs, orders, cmap


def _host_par(B, Pi):
    import ml_dtypes

    Bgm = np.ascontiguousarray(np.transpose(B, (0, 2, 1)).reshape(C, G * M))
    par = np.ascontiguousarray(
        np.concatenate([Bgm, Pi], axis=1).astype(np.float32))
    bpv = Bgm + np.repeat(Pi, M, axis=1)           # (B + Pi) g-major
    bpee = np.zeros((M, G * M + 12 * TCAP), dtype=np.float32)
    bpee[0:C, 0 : G * M] = bpv
    # EE[m', j=(4t+u), 32u+m] = [m' == 11t+m]
    for t in range(NSEC):
        for u in range(4):
            base = G * M + (4 * t + u) * TCAP
            for m in range(R):
                mp = 11 * t + m
                if mp < M:
                    bpee[mp, base + 32 * u + m] = 1.0
    return par, np.ascontiguousarray(bpee.astype(ml_dtypes.bfloat16))


def kernel(x, edge_index, batch, B, Pi):
    from concourse.bass_utils import run_bass_kernel_spmd

    x = np.asarray(x).astype(np.int64)
    batch = np.asarray(batch).astype(np.int64)
    B = np.asarray(B, dtype=np.float32)
    Pi = np.asarray(Pi, dtype=np.float32)

    XLs, XSecs, orders, cmap = _host_pack(x, batch)
    par, bpee = _host_par(B, Pi)

    nc = _build_nc()

    ns_a = WSPLIT * NSEC
    in_maps = [
        {"xl": XLs[i], "xsa": XSecs[i][:, :ns_a], "xsb": XSecs[i][:, ns_a:],
         "par": par, "bpee": bpee}
        for i in range(N_CORES)
    ]

    res = run_bass_kernel_spmd(
        nc, in_maps, core_ids=list(range(N_CORES)), **_RUN_KWARGS
    )
    kernel.last_results = res
    parts = []
    u_of = cmap // NBG
    c_of = cmap % NBG
    for i in range(N_CORES):
        r = res.results[i]["out"]                  # [64, NBG]
        o_sorted = np.empty((GPC, G), dtype=np.float32)
        for gidx in range(G):
            o_sorted[:, gidx] = r[16 * u_of + gidx, c_of]
        o = np.empty_like(o_sorted)
        o[orders[i]] = o_sorted
        parts.append(o)
    out = np.concatenate(parts)
    return out[:, None, :].astype(np.float32)


# test harnesses may set extra run kwargs (e.g. trace) here
_RUN_KWARGS = {}
